# revision 29
# baseline (speedup 1.0000x reference)
"""F0 extractor kernel for trn2 (8 NeuronCores, batch-data-parallel).

Math: for each length-512 frame (hop 256) of the reflect-padded waveform,
f0 = SR / argmax_{p in [32,256)} autocorr(frame, p).  The L2 normalization
in the reference divides every lag of a frame by the same positive scalar,
so it cannot change the argmax and is skipped.

Device pipeline (per core, 8 examples), via autocorr = IDFT(|DFT|^2):
  1. Host pre-transposes the padded signal into 128-sample-block layout
     xb[e, j, g] = xpad[e, 128 g + j] so every DMA row is contiguous;
     per-supertile (64 frames/example) double-buffered SBUF tiles.  The
     four contraction K-tiles of each frame are strided views (frames
     overlap 50%, blocks are stored once).
  2. Forward DFT-767 of every frame as float32r matmuls (1 cycle/row)
     with shared trig weights: X[row, frame] in PSUM; 768 rows = 384 cos
     + 384 sin bins (N odd -> no Nyquist special case).
  3. ScalarE Square into SBUF, VectorE adds Re^2+Im^2 (rows k and 384+k
     are partition-aligned) -> P[bin, frame], 384 rows.
  4. Inverse transform as matmuls: ac[frame, lag] = sum_bin P * C2 with
     P slices stationary so frames land on partitions.  Lag columns
     padded 224->256 (full-rate f32r needs N>=256) with -sum w_k P_k,
     a provable lower bound of every true lag, so pads never win.
  5. VectorE max / max_index straight off PSUM: top-8 values + indices
     per frame -> DRAM.

float32r is TF32-ish: measured end-to-end |approx/N - exact| <= 4.9e-4
of the top-1 scale on this distribution, and the exact argmax always sits
in approx slots 0-1.  The host exactly rescores the top-4 candidate lags
of every frame (fp32 products, fp64 accumulation) and falls back to all
224 lags when the top-4 spread is within 5e-3 of the scale.  Exact-vs-
reference ordering is safe: the top-2 relative gap exceeds 1e-5 on every
frame of this distribution (fp32 reference noise is ~1e-6).
"""

import numpy as np

import concourse.bacc as bacc
import concourse.bass as bass
import concourse.tile as tile
from concourse import mybir
from concourse.bass_utils import run_bass_kernel_spmd

SR = 16000
HOP = 256
FRAME_LEN = 512
PAD = 256
MIN_PERIOD = 32
N_LAGS = 224          # lags 32..255
LAG_COLS = 256        # padded lag columns for full-rate f32r matmul
B = 64
T = 163840
N_FRAMES = 641
N_CORES = 8
EX_PER_CORE = B // N_CORES
T_PAD = T + 2 * PAD            # 164352 = 642 * 256
N_DFT = 767                    # odd: bins 0..383, no Nyquist special case
N_BINS = 384                   # real bins 0..383
ROWS = 768                     # 384 cos rows then 384 sin rows (sin_0 = 0 row)
M_GROUPS = 6                   # 768 / 128 forward output groups
K2_GROUPS = 3                  # 384 power rows / 128 for the inverse matmul
SUP = 64                       # frames per example per supertile
N_SUP = 11                     # 11*64 = 704 >= 641
FR_PAD = N_SUP * SUP           # 704
Y_COLS = FR_PAD + 1            # 705 (phase a//2=1 reads col n+1)
VALID_COLS = T_PAD // HOP      # 642 valid frame columns in Y
N_TILES = N_SUP * 4            # 44 tiles of 128 frames per core

f32 = mybir.dt.float32
f32r = mybir.dt.float32r
u32 = mybir.dt.uint32

_CACHE = {}


def _weights():
    i = np.arange(FRAME_LEN, dtype=np.float64)
    k = np.arange(N_BINS, dtype=np.float64)
    ang = 2.0 * np.pi * np.outer(i, k) / N_DFT            # [512, 384]
    w_fwd = np.concatenate([np.cos(ang), np.sin(ang)], axis=1)            # [512,768]
    # host layout [j, a, m, mb]: i = 128a + j, row = 128m + mb
    wh = (
        w_fwd.reshape(4, 128, M_GROUPS, 128)
        .transpose(1, 0, 2, 3)
        .astype(np.float32)
    )
    wk = np.where(k == 0, 1.0, 2.0)
    p = np.arange(MIN_PERIOD, MIN_PERIOD + N_LAGS, dtype=np.float64)
    c2 = wk[:, None] * np.cos(2.0 * np.pi * np.outer(k, p) / N_DFT)       # [384,224]
    pad = np.repeat(-wk[:, None], LAG_COLS - N_LAGS, axis=1)              # [384,32]
    c2 = np.concatenate([c2, pad], axis=1)                                # [384,256]
    c2h = c2.reshape(K2_GROUPS, 128, LAG_COLS).transpose(1, 0, 2).astype(np.float32)
    return wh, c2h


N_BLOCKS = T_PAD // 128          # 1284 valid 128-blocks per example
G_COLS = 2 * Y_COLS              # 1410 block columns incl. zero pad


def _build_nc():
    nc = bacc.Bacc("TRN2", target_bir_lowering=False, debug=False, num_devices=1)
    x = nc.dram_tensor("xb", [EX_PER_CORE, 128, G_COLS], f32r, kind="ExternalInput").ap()
    wdft = nc.dram_tensor("wdft", [128, 4, M_GROUPS, 128], f32r, kind="ExternalInput").ap()
    c2 = nc.dram_tensor("c2", [128, K2_GROUPS, LAG_COLS], f32r, kind="ExternalInput").ap()
    idx_out = nc.dram_tensor("idx", [128, N_TILES, 8], u32, kind="ExternalOutput").ap()
    val_out = nc.dram_tensor("val", [128, N_TILES, 8], f32, kind="ExternalOutput").ap()

    with tile.TileContext(nc) as tc:
        with (
            tc.tile_pool(name="singles", bufs=1) as singles,
            tc.tile_pool(name="ypool", bufs=3) as ypool,
            tc.tile_pool(name="ppool", bufs=3) as ppool,
            tc.tile_pool(name="psum1", bufs=5, space="PSUM") as psum1,
            tc.tile_pool(name="psum2", bufs=3, space="PSUM") as psum2,
        ):
            # DMA issue order = first-use order: supertile-0 signal, then the
            # six forward-weight chunks, then the inverse weights.
            GS = 2 * SUP + 2          # 130 block columns per supertile

            def y_dma(pool, s):
                y_s = pool.tile([128, EX_PER_CORE, GS], f32r, tag="ys")
                src = bass.AP(
                    tensor=x.tensor,
                    offset=128 * s,
                    ap=[[G_COLS, 128], [128 * G_COLS, EX_PER_CORE], [1, GS]],
                )
                nc.sync.dma_start(out=y_s, in_=src)
                return y_s

            y_first = y_dma(ypool, 0)
            w_sb = singles.tile([128, 4, M_GROUPS, 128], f32r, tag="w")
            c2_sb = singles.tile([128, K2_GROUPS, LAG_COLS], f32r, tag="c2")
            for m in range(M_GROUPS):
                nc.sync.dma_start(out=w_sb[:, :, m, :], in_=wdft[:, :, m, :])
            nc.sync.dma_start(out=c2_sb, in_=c2)

            collect_i = singles.tile([128, N_TILES, 8], u32, tag="ci")
            collect_v = singles.tile([128, N_TILES, 8], f32, tag="cv")

            # Signal in block layout (host pre-transposed): xb[e, j, g] =
            # xpad[e, 128g + j]; per-supertile double-buffered tiles with
            # per-partition contiguous DMA rows.
            for s in range(N_SUP):
                y_s = y_first if s == 0 else y_dma(ypool, s)
                # g = 2m + r: frame n at phase a reads (m = n - 64 s + a//2, r = a%2)
                yv = y_s.rearrange("p e (m r) -> p e m r", r=2)
                sqs = []
                for m in range(M_GROUPS):
                    x_ps = psum1.tile([128, EX_PER_CORE, SUP], f32)
                    for a in range(4):
                        off = a // 2
                        rhs = yv[:, :, off : off + SUP, a % 2]
                        nc.tensor.matmul(
                            x_ps,
                            w_sb[:, a, m, :],
                            rhs,
                            start=(a == 0),
                            stop=(a == 3),
                        )
                    sq = ppool.tile([128, EX_PER_CORE, SUP], f32, tag=f"sq{m}")
                    nc.scalar.square(sq, x_ps)
                    sqs.append(sq)
                ps = []
                for m in range(K2_GROUPS):
                    p_t = ppool.tile([128, EX_PER_CORE, SUP], f32r, tag=f"p{m}")
                    nc.vector.tensor_add(p_t, sqs[m], sqs[m + K2_GROUPS])
                    ps.append(p_t)
                for c in range(4):
                    ac_ps = psum2.tile([128, LAG_COLS], f32)
                    for m in range(K2_GROUPS):
                        nc.tensor.matmul(
                            ac_ps,
                            ps[m][:, 2 * c : 2 * (c + 1), :],
                            c2_sb[:, m, :],
                            start=(m == 0),
                            stop=(m == K2_GROUPS - 1),
                        )
                    t = 4 * s + c
                    nc.vector.max(collect_v[:, t, :], ac_ps)
                    nc.vector.max_index(collect_i[:, t, :], collect_v[:, t, :], ac_ps)

            half = N_TILES // 2
            nc.sync.dma_start(out=idx_out[:, :half], in_=collect_i[:, :half])
            nc.sync.dma_start(out=val_out[:, :half], in_=collect_v[:, :half])
            nc.sync.dma_start(out=idx_out[:, half:], in_=collect_i[:, half:])
            nc.sync.dma_start(out=val_out[:, half:], in_=collect_v[:, half:])
    nc.compile()
    return nc


def _get_nc():
    if "nc" not in _CACHE:
        _CACHE["nc"] = _build_nc()
        _CACHE["w"] = _weights()
    return _CACHE["nc"]


def modeled_exec_ns():
    """Per-core kernel time from the instruction cost model (TimelineSim).
    The axon client in this container has no NTFF profiling hook, so this
    is the best available device-time estimate."""
    from concourse import timeline_sim as ts

    class _Null:
        def __getattr__(self, name):
            return lambda *a, **k: None

    orig = ts._build_perfetto
    ts._build_perfetto = lambda core_id: _Null()
    try:
        return int(ts.TimelineSim(_get_nc(), trace=False).simulate())
    finally:
        ts._build_perfetto = orig


def _device_topk(xpad):
    """xpad: (64, T_PAD) fp32 -> (idx8, val8): (64, 641, 8) candidate lags/values."""
    nc = _get_nc()
    wh, c2h = _CACHE["w"]
    # block-transposed layout: xb[e, j, g] = xpad[e, 128 g + j], zero-padded
    xb = np.zeros((B, 128, G_COLS), dtype=np.float32)
    xb[:, :, :N_BLOCKS] = xpad.reshape(B, N_BLOCKS, 128).transpose(0, 2, 1)
    in_maps = []
    for r in range(N_CORES):
        in_maps.append(
            {
                "xb": np.ascontiguousarray(xb[r * EX_PER_CORE : (r + 1) * EX_PER_CORE]),
                "wdft": wh,
                "c2": c2h,
            }
        )
    trace = bool(int(__import__("os").environ.get("F0_TRACE", "0")))
    res = None
    for attempt in range(3):
        try:
            res = run_bass_kernel_spmd(nc, in_maps, list(range(N_CORES)), trace=trace)
            break
        except Exception:
            # transient NRT device errors have been observed; retry
            if attempt == 2:
                raise
    _CACHE["last_exec_time_ns"] = res.exec_time_ns
    idx8 = np.empty((B, FR_PAD, 8), dtype=np.int64)
    val8 = np.empty((B, FR_PAD, 8), dtype=np.float32)
    for r in range(N_CORES):
        # device arrays [128 q, 44 t, 8]; q -> (e2, qq), t = 4s + c,
        # example e = 2c + e2, frame n = 64s + qq
        di = res.results[r]["idx"].reshape(2, 64, N_SUP, 4, 8)
        dv = res.results[r]["val"].reshape(2, 64, N_SUP, 4, 8)
        idx8[r * 8 : (r + 1) * 8] = (
            di.transpose(3, 0, 2, 1, 4).reshape(8, FR_PAD, 8).astype(np.int64)
        )
        val8[r * 8 : (r + 1) * 8] = dv.transpose(3, 0, 2, 1, 4).reshape(8, FR_PAD, 8)
    return idx8[:, :N_FRAMES], val8[:, :N_FRAMES]


N_SLOTS = 4        # candidate lags rescored exactly per frame (of 8 returned)


def _exact_rescore(xpad, idx_slots):
    """Exact autocorrelation at the candidate lags: fp32 products (matching
    the reference's own fp32 product rounding scale), fp64 accumulation."""
    nb, nf, ns = idx_slots.shape
    starts = np.arange(nf) * HOP
    frames = np.lib.stride_tricks.sliding_window_view(xpad, FRAME_LEN, axis=1)[
        :, starts
    ]                                                     # (B, F, 512) fp32 view
    fpad = np.concatenate(
        [frames, np.zeros((nb, nf, FRAME_LEN), np.float32)], axis=2
    )                                                     # (B, F, 1024)
    lags = (idx_slots + MIN_PERIOD).astype(np.int32)      # (B, F, ns)
    i = np.arange(FRAME_LEN, dtype=np.int32)
    exact = np.empty(lags.shape, dtype=np.float64)
    for r in range(ns):
        shifted = np.take_along_axis(fpad, i + lags[:, :, r : r + 1], axis=2)
        exact[:, :, r] = (frames * shifted).sum(axis=2, dtype=np.float64)
    return exact


def _full_rescore(xpad, rows_b, rows_f):
    """All-224-lag exact autocorrelation argmax for ambiguous frames."""
    fr = np.stack(
        [xpad[b_, f_ * HOP : f_ * HOP + FRAME_LEN] for b_, f_ in zip(rows_b, rows_f)]
    ).astype(np.float64)                                  # (R, 512)
    ac = np.empty((len(rows_b), N_LAGS))
    for j, p in enumerate(range(MIN_PERIOD, 256)):
        ac[:, j] = np.einsum("ri,ri->r", fr[:, : FRAME_LEN - p], fr[:, p:])
    return np.argmax(ac, axis=1).astype(np.int64)


def kernel(waveform):
    waveform = np.asarray(waveform, dtype=np.float32)
    x = waveform[:, 0, :]
    xpad = np.pad(x, ((0, 0), (PAD, PAD)), mode="reflect")
    idx8, val8 = _device_topk(xpad)

    idx4 = idx8[:, :, :N_SLOTS]
    exact = _exact_rescore(xpad, idx4)
    # among the candidates pick the exact-max; ties -> smallest lag
    order = np.argsort(idx4, axis=2)                       # evaluate in lag order
    exact_sorted = np.take_along_axis(exact, order, axis=2)
    idx_sorted = np.take_along_axis(idx4, order, axis=2)
    best_slot = np.argmax(exact_sorted, axis=2)            # first max in lag order
    best_idx = np.take_along_axis(idx_sorted, best_slot[..., None], axis=2)[..., 0]

    # Frames where the approximate top-4 window may not contain the true
    # argmax: approximate spread below 10x the measured f32r error bound
    # (end-to-end |approx/N - exact| <= 4.9e-4 * top1 scale on this
    # distribution) -> exact argmax over all 224 lags instead.
    scale = np.abs(val8[:, :, 0]) + 1e-20
    spread = val8[:, :, 0] - val8[:, :, N_SLOTS - 1]
    risky = spread < 5e-3 * scale
    if np.any(risky):
        rb, rf = np.nonzero(risky)
        best_idx[rb, rf] = _full_rescore(xpad, rb, rf)

    period = best_idx.astype(np.float32) + np.float32(MIN_PERIOD)
    f0 = np.float32(SR) / (period + np.float32(1e-8))
    return np.clip(f0, np.float32(50.0), np.float32(500.0)).astype(np.float32)


# revision 37
# speedup vs baseline: 1.0611x; 1.0611x over previous
"""F0 extractor kernel for trn2 (8 NeuronCores, batch-data-parallel).

Math: for each length-512 frame (hop 256) of the reflect-padded waveform,
f0 = SR / argmax_{p in [32,256)} autocorr(frame, p).  The L2 normalization
in the reference divides every lag of a frame by the same positive scalar,
so it cannot change the argmax and is skipped.

Device pipeline (per core, 8 examples), via autocorr = IDFT(|DFT|^2):
  1. Host pre-transposes the padded signal into 128-sample-block layout
     xb[e, j, g] = xpad[e, 128 g + j] so every DMA row is contiguous;
     per-supertile (64 frames/example) double-buffered SBUF tiles.  The
     four contraction K-tiles of each frame are strided views (frames
     overlap 50%, blocks are stored once).
  2. Forward DFT-767 of every frame as float32r matmuls (1 cycle/row)
     with shared trig weights: X[row, frame] in PSUM; 768 rows = 384 cos
     + 384 sin bins (N odd -> no Nyquist special case).
  3. ScalarE Square into SBUF, VectorE adds Re^2+Im^2 (rows k and 384+k
     are partition-aligned) -> P[bin, frame], 384 rows.
  4. Inverse transform as matmuls: ac[frame, lag] = sum_bin P * C2 with
     P slices stationary so frames land on partitions.  Lag columns
     padded 224->256 (full-rate f32r needs N>=256) with -sum w_k P_k,
     a provable lower bound of every true lag, so pads never win.
  5. VectorE max / max_index straight off PSUM: top-8 values + indices
     per frame -> DRAM.

float32r is TF32-ish: measured end-to-end |approx/N - exact| <= 4.9e-4
of the top-1 scale on this distribution, and the exact argmax always sits
in approx slots 0-1.  The host exactly rescores the top-4 candidate lags
of every frame (fp32 products, fp64 accumulation) and falls back to all
224 lags when the top-4 spread is within 5e-3 of the scale.  Exact-vs-
reference ordering is safe: the top-2 relative gap exceeds 1e-5 on every
frame of this distribution (fp32 reference noise is ~1e-6).
"""

import numpy as np

import concourse.bacc as bacc
import concourse.bass as bass
import concourse.tile as tile
from concourse import mybir
from concourse.bass_utils import run_bass_kernel_spmd

SR = 16000
HOP = 256
FRAME_LEN = 512
PAD = 256
MIN_PERIOD = 32
N_LAGS = 224          # lags 32..255
LAG_COLS = 256        # padded lag columns for full-rate f32r matmul
B = 64
T = 163840
N_FRAMES = 641
N_CORES = 8
EX_PER_CORE = B // N_CORES
T_PAD = T + 2 * PAD            # 164352 = 642 * 256
N_DFT = 767                    # odd: bins 0..383, no Nyquist special case
N_BINS = 384                   # real bins 0..383
ROWS = 768                     # 384 cos rows then 384 sin rows (sin_0 = 0 row)
M_GROUPS = 6                   # 768 / 128 forward output groups
K2_GROUPS = 3                  # 384 power rows / 128 for the inverse matmul
SUP = 64                       # frames per example per supertile
N_SUP = 10                     # frames 0..639; frame 640 via a cleanup pass
N_TILES = N_SUP * 4            # 40 tiles of 128 frames per core

f32 = mybir.dt.float32
f32r = mybir.dt.float32r
u32 = mybir.dt.uint32

_CACHE = {}


def _weights():
    i = np.arange(FRAME_LEN, dtype=np.float64)
    k = np.arange(N_BINS, dtype=np.float64)
    ang = 2.0 * np.pi * np.outer(i, k) / N_DFT            # [512, 384]
    w_fwd = np.concatenate([np.cos(ang), np.sin(ang)], axis=1)            # [512,768]
    # host layout [j, a, m, mb]: i = 128a + j, row = 128m + mb
    wh = (
        w_fwd.reshape(4, 128, M_GROUPS, 128)
        .transpose(1, 0, 2, 3)
        .astype(np.float32)
    )
    wk = np.where(k == 0, 1.0, 2.0)
    p = np.arange(MIN_PERIOD, MIN_PERIOD + N_LAGS, dtype=np.float64)
    c2 = wk[:, None] * np.cos(2.0 * np.pi * np.outer(k, p) / N_DFT)       # [384,224]
    pad = np.repeat(-wk[:, None], LAG_COLS - N_LAGS, axis=1)              # [384,32]
    c2 = np.concatenate([c2, pad], axis=1)                                # [384,256]
    c2h = c2.reshape(K2_GROUPS, 128, LAG_COLS).transpose(1, 0, 2).astype(np.float32)
    return wh, c2h


N_BLOCKS = T_PAD // 128          # 1284 128-blocks per example (no padding)
G_COLS = N_BLOCKS


def _build_nc():
    nc = bacc.Bacc("TRN2", target_bir_lowering=False, debug=False, num_devices=1)
    x = nc.dram_tensor("xb", [EX_PER_CORE, 128, G_COLS], f32r, kind="ExternalInput").ap()
    wdft = nc.dram_tensor("wdft", [128, 4, M_GROUPS, 128], f32r, kind="ExternalInput").ap()
    c2 = nc.dram_tensor("c2", [128, K2_GROUPS, LAG_COLS], f32r, kind="ExternalInput").ap()
    idx_out = nc.dram_tensor("idx", [128, N_TILES, 8], u32, kind="ExternalOutput").ap()
    val_out = nc.dram_tensor("val", [128, N_TILES, 8], f32, kind="ExternalOutput").ap()
    idx_l = nc.dram_tensor("idx_l", [EX_PER_CORE, 8], u32, kind="ExternalOutput").ap()
    val_l = nc.dram_tensor("val_l", [EX_PER_CORE, 8], f32, kind="ExternalOutput").ap()

    with tile.TileContext(nc) as tc:
        with (
            tc.tile_pool(name="singles", bufs=1) as singles,
            tc.tile_pool(name="ypool", bufs=3) as ypool,
            tc.tile_pool(name="ppool", bufs=3) as ppool,
            tc.tile_pool(name="psum1", bufs=5, space="PSUM") as psum1,
            tc.tile_pool(name="psum2", bufs=3, space="PSUM") as psum2,
        ):
            # DMA issue order = first-use order: supertile-0 signal, then the
            # six forward-weight chunks, then the inverse weights.
            GS = 2 * SUP + 2          # 130 block columns per supertile

            def y_dma(pool, s):
                y_s = pool.tile([128, EX_PER_CORE, GS], f32r, tag="ys")
                src = bass.AP(
                    tensor=x.tensor,
                    offset=128 * s,
                    ap=[[G_COLS, 128], [128 * G_COLS, EX_PER_CORE], [1, GS]],
                )
                nc.sync.dma_start(out=y_s, in_=src)
                return y_s

            y_first = y_dma(ypool, 0)
            w_sb = singles.tile([128, 4, M_GROUPS, 128], f32r, tag="w")
            c2_sb = singles.tile([128, K2_GROUPS, LAG_COLS], f32r, tag="c2")
            for m in range(M_GROUPS):
                nc.sync.dma_start(out=w_sb[:, :, m, :], in_=wdft[:, :, m, :])
            nc.sync.dma_start(out=c2_sb, in_=c2)

            collect_i = singles.tile([128, N_TILES, 8], u32, tag="ci")
            collect_v = singles.tile([128, N_TILES, 8], f32, tag="cv")

            # Signal in block layout (host pre-transposed): xb[e, j, g] =
            # xpad[e, 128g + j]; per-supertile double-buffered tiles with
            # per-partition contiguous DMA rows.
            for s in range(N_SUP):
                y_s = y_first if s == 0 else y_dma(ypool, s)
                # g = 2m + r: frame n at phase a reads (m = n - 64 s + a//2, r = a%2)
                yv = y_s.rearrange("p e (m r) -> p e m r", r=2)
                sqs = []
                for m in range(M_GROUPS):
                    x_ps = psum1.tile([128, EX_PER_CORE, SUP], f32)
                    for a in range(4):
                        off = a // 2
                        rhs = yv[:, :, off : off + SUP, a % 2]
                        nc.tensor.matmul(
                            x_ps,
                            w_sb[:, a, m, :],
                            rhs,
                            start=(a == 0),
                            stop=(a == 3),
                        )
                    sq = ppool.tile([128, EX_PER_CORE, SUP], f32, tag=f"sq{m}")
                    nc.scalar.square(sq, x_ps)
                    sqs.append(sq)
                ps = []
                for m in range(K2_GROUPS):
                    p_t = ppool.tile([128, EX_PER_CORE, SUP], f32r, tag=f"p{m}")
                    nc.vector.tensor_add(p_t, sqs[m], sqs[m + K2_GROUPS])
                    ps.append(p_t)
                for c in range(4):
                    ac_ps = psum2.tile([128, LAG_COLS], f32)
                    for m in range(K2_GROUPS):
                        nc.tensor.matmul(
                            ac_ps,
                            ps[m][:, 2 * c : 2 * (c + 1), :],
                            c2_sb[:, m, :],
                            start=(m == 0),
                            stop=(m == K2_GROUPS - 1),
                        )
                    t = 4 * s + c
                    nc.vector.max(collect_v[:, t, :], ac_ps)
                    nc.vector.max_index(collect_i[:, t, :], collect_v[:, t, :], ac_ps)

            half = N_TILES // 2
            nc.sync.dma_start(out=idx_out[:, :half], in_=collect_i[:, :half])
            nc.sync.dma_start(out=val_out[:, :half], in_=collect_v[:, :half])

            # cleanup pass: frame 640 of each example (blocks 1280..1283)
            y_l = singles.tile([128, EX_PER_CORE, 4], f32r, tag="yl")
            src = bass.AP(
                tensor=x.tensor,
                offset=2 * N_SUP * SUP,
                ap=[[G_COLS, 128], [128 * G_COLS, EX_PER_CORE], [1, 4]],
            )
            nc.sync.dma_start(out=y_l, in_=src)
            yvl = y_l.rearrange("p e (m r) -> p e m r", r=2)
            sqs = []
            for m in range(M_GROUPS):
                x_ps = psum1.tile([128, EX_PER_CORE], f32)
                for a in range(4):
                    rhs = yvl[:, :, a // 2, a % 2]
                    nc.tensor.matmul(
                        x_ps, w_sb[:, a, m, :], rhs, start=(a == 0), stop=(a == 3)
                    )
                sq = ppool.tile([128, EX_PER_CORE], f32, tag=f"sql{m}")
                nc.scalar.square(sq, x_ps)
                sqs.append(sq)
            ps = []
            for m in range(K2_GROUPS):
                p_t = ppool.tile([128, EX_PER_CORE], f32r, tag=f"pl{m}")
                nc.vector.tensor_add(p_t, sqs[m], sqs[m + K2_GROUPS])
                ps.append(p_t)
            ac_ps = psum2.tile([EX_PER_CORE, LAG_COLS], f32)
            for m in range(K2_GROUPS):
                nc.tensor.matmul(
                    ac_ps, ps[m], c2_sb[:, m, :],
                    start=(m == 0), stop=(m == K2_GROUPS - 1),
                )
            vl = singles.tile([EX_PER_CORE, 8], f32, tag="vl")
            il = singles.tile([EX_PER_CORE, 8], u32, tag="il")
            nc.vector.max(vl, ac_ps)
            nc.vector.max_index(il, vl, ac_ps)
            nc.sync.dma_start(out=val_l, in_=vl)
            nc.sync.dma_start(out=idx_l, in_=il)

            nc.sync.dma_start(out=idx_out[:, half:], in_=collect_i[:, half:])
            nc.sync.dma_start(out=val_out[:, half:], in_=collect_v[:, half:])
    nc.compile()
    return nc


def _get_nc():
    if "nc" not in _CACHE:
        _CACHE["nc"] = _build_nc()
        _CACHE["w"] = _weights()
    return _CACHE["nc"]


def modeled_exec_ns():
    """Per-core kernel time from the instruction cost model (TimelineSim).
    The axon client in this container has no NTFF profiling hook, so this
    is the best available device-time estimate."""
    from concourse import timeline_sim as ts

    class _Null:
        def __getattr__(self, name):
            return lambda *a, **k: None

    orig = ts._build_perfetto
    ts._build_perfetto = lambda core_id: _Null()
    try:
        return int(ts.TimelineSim(_get_nc(), trace=False).simulate())
    finally:
        ts._build_perfetto = orig


def _device_topk(xpad):
    """xpad: (64, T_PAD) fp32 -> (idx8, val8): (64, 641, 8) candidate lags/values."""
    nc = _get_nc()
    wh, c2h = _CACHE["w"]
    # block-transposed layout: xb[e, j, g] = xpad[e, 128 g + j]
    xb = np.ascontiguousarray(xpad.reshape(B, N_BLOCKS, 128).transpose(0, 2, 1))
    in_maps = []
    for r in range(N_CORES):
        in_maps.append(
            {
                "xb": np.ascontiguousarray(xb[r * EX_PER_CORE : (r + 1) * EX_PER_CORE]),
                "wdft": wh,
                "c2": c2h,
            }
        )
    trace = bool(int(__import__("os").environ.get("F0_TRACE", "0")))
    res = None
    for attempt in range(3):
        try:
            res = run_bass_kernel_spmd(nc, in_maps, list(range(N_CORES)), trace=trace)
            break
        except Exception:
            # transient NRT device errors have been observed; retry
            if attempt == 2:
                raise
    _CACHE["last_exec_time_ns"] = res.exec_time_ns
    idx8 = np.empty((B, N_FRAMES, 8), dtype=np.int64)
    val8 = np.empty((B, N_FRAMES, 8), dtype=np.float32)
    nmain = N_SUP * SUP
    for r in range(N_CORES):
        # device arrays [128 q, 40 t, 8]; q -> (e2, qq), t = 4s + c,
        # example e = 2c + e2, frame n = 64s + qq; frame 640 from idx_l/val_l
        di = res.results[r]["idx"].reshape(2, 64, N_SUP, 4, 8)
        dv = res.results[r]["val"].reshape(2, 64, N_SUP, 4, 8)
        sl = slice(r * EX_PER_CORE, (r + 1) * EX_PER_CORE)
        idx8[sl, :nmain] = (
            di.transpose(3, 0, 2, 1, 4).reshape(EX_PER_CORE, nmain, 8)
        )
        val8[sl, :nmain] = dv.transpose(3, 0, 2, 1, 4).reshape(EX_PER_CORE, nmain, 8)
        idx8[sl, nmain] = res.results[r]["idx_l"]
        val8[sl, nmain] = res.results[r]["val_l"]
    return idx8, val8


N_SLOTS = 4        # candidate lags rescored exactly per frame (of 8 returned)


def _exact_rescore(xpad, idx_slots):
    """Exact autocorrelation at the candidate lags: fp32 products (matching
    the reference's own fp32 product rounding scale), fp64 accumulation."""
    nb, nf, ns = idx_slots.shape
    starts = np.arange(nf) * HOP
    frames = np.lib.stride_tricks.sliding_window_view(xpad, FRAME_LEN, axis=1)[
        :, starts
    ]                                                     # (B, F, 512) fp32 view
    fpad = np.concatenate(
        [frames, np.zeros((nb, nf, FRAME_LEN), np.float32)], axis=2
    )                                                     # (B, F, 1024)
    lags = (idx_slots + MIN_PERIOD).astype(np.int32)      # (B, F, ns)
    i = np.arange(FRAME_LEN, dtype=np.int32)
    exact = np.empty(lags.shape, dtype=np.float64)
    for r in range(ns):
        shifted = np.take_along_axis(fpad, i + lags[:, :, r : r + 1], axis=2)
        exact[:, :, r] = (frames * shifted).sum(axis=2, dtype=np.float64)
    return exact


def _full_rescore(xpad, rows_b, rows_f):
    """All-224-lag exact autocorrelation argmax for ambiguous frames."""
    fr = np.stack(
        [xpad[b_, f_ * HOP : f_ * HOP + FRAME_LEN] for b_, f_ in zip(rows_b, rows_f)]
    ).astype(np.float64)                                  # (R, 512)
    ac = np.empty((len(rows_b), N_LAGS))
    for j, p in enumerate(range(MIN_PERIOD, 256)):
        ac[:, j] = np.einsum("ri,ri->r", fr[:, : FRAME_LEN - p], fr[:, p:])
    return np.argmax(ac, axis=1).astype(np.int64)


def kernel(waveform):
    waveform = np.asarray(waveform, dtype=np.float32)
    x = waveform[:, 0, :]
    xpad = np.pad(x, ((0, 0), (PAD, PAD)), mode="reflect")
    idx8, val8 = _device_topk(xpad)

    idx4 = idx8[:, :, :N_SLOTS]
    exact = _exact_rescore(xpad, idx4)
    # among the candidates pick the exact-max; ties -> smallest lag
    order = np.argsort(idx4, axis=2)                       # evaluate in lag order
    exact_sorted = np.take_along_axis(exact, order, axis=2)
    idx_sorted = np.take_along_axis(idx4, order, axis=2)
    best_slot = np.argmax(exact_sorted, axis=2)            # first max in lag order
    best_idx = np.take_along_axis(idx_sorted, best_slot[..., None], axis=2)[..., 0]

    # Frames where the approximate top-4 window may not contain the true
    # argmax: approximate spread below 10x the measured f32r error bound
    # (end-to-end |approx/N - exact| <= 4.9e-4 * top1 scale on this
    # distribution) -> exact argmax over all 224 lags instead.
    scale = np.abs(val8[:, :, 0]) + 1e-20
    spread = val8[:, :, 0] - val8[:, :, N_SLOTS - 1]
    risky = spread < 5e-3 * scale
    if np.any(risky):
        rb, rf = np.nonzero(risky)
        best_idx[rb, rf] = _full_rescore(xpad, rb, rf)

    period = best_idx.astype(np.float32) + np.float32(MIN_PERIOD)
    f0 = np.float32(SR) / (period + np.float32(1e-8))
    return np.clip(f0, np.float32(50.0), np.float32(500.0)).astype(np.float32)


# revision 39
# speedup vs baseline: 1.0658x; 1.0044x over previous
"""F0 extractor kernel for trn2 (8 NeuronCores, batch-data-parallel).

Math: for each length-512 frame (hop 256) of the reflect-padded waveform,
f0 = SR / argmax_{p in [32,256)} autocorr(frame, p).  The L2 normalization
in the reference divides every lag of a frame by the same positive scalar,
so it cannot change the argmax and is skipped.

Device pipeline (per core, 8 examples), via autocorr = IDFT(|DFT|^2):
  1. Host pre-transposes the padded signal into 128-sample-block layout
     xb[e, j, g] = xpad[e, 128 g + j] so every DMA row is contiguous;
     per-supertile (64 frames/example) double-buffered SBUF tiles.  The
     four contraction K-tiles of each frame are strided views (frames
     overlap 50%, blocks are stored once).
  2. Forward DFT-767 of every frame as float32r matmuls (1 cycle/row)
     with shared trig weights: X[row, frame] in PSUM; 768 rows = 384 cos
     + 384 sin bins (N odd -> no Nyquist special case).
  3. ScalarE Square into SBUF, VectorE adds Re^2+Im^2 (rows k and 384+k
     are partition-aligned) -> P[bin, frame], 384 rows.
  4. Inverse transform as matmuls: ac[frame, lag] = sum_bin P * C2 with
     P slices stationary so frames land on partitions.  Lag columns
     padded 224->256 (full-rate f32r needs N>=256) with -sum w_k P_k,
     a provable lower bound of every true lag, so pads never win.
  5. VectorE max / max_index straight off PSUM: top-8 values + indices
     per frame -> DRAM.

float32r is TF32-ish: measured end-to-end |approx/N - exact| <= 4.9e-4
of the top-1 scale on this distribution, and the exact argmax always sits
in approx slots 0-1.  The host exactly rescores the top-4 candidate lags
of every frame (fp32 products, fp64 accumulation) and falls back to all
224 lags when the top-4 spread is within 5e-3 of the scale.  Exact-vs-
reference ordering is safe: the top-2 relative gap exceeds 1e-5 on every
frame of this distribution (fp32 reference noise is ~1e-6).
"""

import numpy as np

import concourse.bacc as bacc
import concourse.bass as bass
import concourse.tile as tile
from concourse import mybir
from concourse.bass_utils import run_bass_kernel_spmd

SR = 16000
HOP = 256
FRAME_LEN = 512
PAD = 256
MIN_PERIOD = 32
N_LAGS = 224          # lags 32..255
LAG_COLS = 256        # padded lag columns for full-rate f32r matmul
B = 64
T = 163840
N_FRAMES = 641
N_CORES = 8
EX_PER_CORE = B // N_CORES
T_PAD = T + 2 * PAD            # 164352 = 642 * 256
N_DFT = 767                    # odd: bins 0..383, no Nyquist special case
N_BINS = 384                   # real bins 0..383
ROWS = 768                     # 384 cos rows then 384 sin rows (sin_0 = 0 row)
M_GROUPS = 6                   # 768 / 128 forward output groups
K2_GROUPS = 3                  # 384 power rows / 128 for the inverse matmul
SUP = 64                       # frames per example per supertile
N_SUP = 10                     # frames 0..639; frame 640 via a cleanup pass
N_TILES = N_SUP * 4            # 40 tiles of 128 frames per core

f32 = mybir.dt.float32
f32r = mybir.dt.float32r
u32 = mybir.dt.uint32

_CACHE = {}


def _weights():
    i = np.arange(FRAME_LEN, dtype=np.float64)
    k = np.arange(N_BINS, dtype=np.float64)
    ang = 2.0 * np.pi * np.outer(i, k) / N_DFT            # [512, 384]
    w_fwd = np.concatenate([np.cos(ang), np.sin(ang)], axis=1)            # [512,768]
    # host layout [j, a, m, mb]: i = 128a + j, row = 128m + mb
    wh = (
        w_fwd.reshape(4, 128, M_GROUPS, 128)
        .transpose(1, 0, 2, 3)
        .astype(np.float32)
    )
    wk = np.where(k == 0, 1.0, 2.0)
    p = np.arange(MIN_PERIOD, MIN_PERIOD + N_LAGS, dtype=np.float64)
    c2 = wk[:, None] * np.cos(2.0 * np.pi * np.outer(k, p) / N_DFT)       # [384,224]
    pad = np.repeat(-wk[:, None], LAG_COLS - N_LAGS, axis=1)              # [384,32]
    c2 = np.concatenate([c2, pad], axis=1)                                # [384,256]
    c2h = c2.reshape(K2_GROUPS, 128, LAG_COLS).transpose(1, 0, 2).astype(np.float32)
    return wh, c2h


N_BLOCKS = T_PAD // 128          # 1284 128-blocks per example (no padding)
G_COLS = N_BLOCKS


def _build_nc():
    nc = bacc.Bacc("TRN2", target_bir_lowering=False, debug=False, num_devices=1)
    x = nc.dram_tensor("xb", [EX_PER_CORE, 128, G_COLS], f32r, kind="ExternalInput").ap()
    wdft = nc.dram_tensor("wdft", [128, 4, M_GROUPS, 128], f32r, kind="ExternalInput").ap()
    c2 = nc.dram_tensor("c2", [128, K2_GROUPS, LAG_COLS], f32r, kind="ExternalInput").ap()
    idx_out = nc.dram_tensor("idx", [128, N_TILES, 8], u32, kind="ExternalOutput").ap()
    val_out = nc.dram_tensor("val", [128, N_TILES, 8], f32, kind="ExternalOutput").ap()
    idx_l = nc.dram_tensor("idx_l", [EX_PER_CORE, 8], u32, kind="ExternalOutput").ap()
    val_l = nc.dram_tensor("val_l", [EX_PER_CORE, 8], f32, kind="ExternalOutput").ap()

    with tile.TileContext(nc) as tc:
        with (
            tc.tile_pool(name="singles", bufs=1) as singles,
            tc.tile_pool(name="ypool", bufs=3) as ypool,
            tc.tile_pool(name="ppool", bufs=3) as ppool,
            tc.tile_pool(name="psum1", bufs=5, space="PSUM") as psum1,
            tc.tile_pool(name="psum2", bufs=3, space="PSUM") as psum2,
        ):
            # DMA issue order = first-use order: supertile-0 signal, then the
            # six forward-weight chunks, then the inverse weights.
            GS = 2 * SUP + 2          # 130 block columns per supertile

            def y_dma(pool, s):
                y_s = pool.tile([128, EX_PER_CORE, GS], f32r, tag="ys")
                src = bass.AP(
                    tensor=x.tensor,
                    offset=128 * s,
                    ap=[[G_COLS, 128], [128 * G_COLS, EX_PER_CORE], [1, GS]],
                )
                nc.sync.dma_start(out=y_s, in_=src)
                return y_s

            y_first = y_dma(ypool, 0)
            w_sb = singles.tile([128, 4, M_GROUPS, 128], f32r, tag="w")
            c2_sb = singles.tile([128, K2_GROUPS, LAG_COLS], f32r, tag="c2")
            for m in range(M_GROUPS):
                nc.sync.dma_start(out=w_sb[:, :, m, :], in_=wdft[:, :, m, :])
            nc.sync.dma_start(out=c2_sb, in_=c2)


            collect_i = singles.tile([128, N_TILES, 8], u32, tag="ci")
            collect_v = singles.tile([128, N_TILES, 8], f32, tag="cv")

            def cleanup_pass():
                # cleanup pass: frame 640 of each example (blocks 1280..1283)
                y_l = singles.tile([128, EX_PER_CORE, 4], f32r, tag="yl")
                src = bass.AP(
                    tensor=x.tensor,
                    offset=2 * N_SUP * SUP,
                    ap=[[G_COLS, 128], [128 * G_COLS, EX_PER_CORE], [1, 4]],
                )
                nc.sync.dma_start(out=y_l, in_=src)
                yvl = y_l.rearrange("p e (m r) -> p e m r", r=2)
                sqs = []
                for m in range(M_GROUPS):
                    x_ps = psum1.tile([128, EX_PER_CORE], f32)
                    for a in range(4):
                        rhs = yvl[:, :, a // 2, a % 2]
                        nc.tensor.matmul(
                            x_ps, w_sb[:, a, m, :], rhs, start=(a == 0), stop=(a == 3)
                        )
                    sq = ppool.tile([128, EX_PER_CORE], f32, tag=f"sql{m}")
                    nc.scalar.square(sq, x_ps)
                    sqs.append(sq)
                ps = []
                for m in range(K2_GROUPS):
                    p_t = ppool.tile([128, EX_PER_CORE], f32r, tag=f"pl{m}")
                    nc.vector.tensor_add(p_t, sqs[m], sqs[m + K2_GROUPS])
                    ps.append(p_t)
                ac_ps = psum2.tile([EX_PER_CORE, LAG_COLS], f32)
                for m in range(K2_GROUPS):
                    nc.tensor.matmul(
                        ac_ps, ps[m], c2_sb[:, m, :],
                        start=(m == 0), stop=(m == K2_GROUPS - 1),
                    )
                vl = singles.tile([EX_PER_CORE, 8], f32, tag="vl")
                il = singles.tile([EX_PER_CORE, 8], u32, tag="il")
                nc.vector.max(vl, ac_ps)
                nc.vector.max_index(il, vl, ac_ps)
                nc.sync.dma_start(out=val_l, in_=vl)
                nc.sync.dma_start(out=idx_l, in_=il)

            # Signal in block layout (host pre-transposed): xb[e, j, g] =
            # xpad[e, 128g + j]; per-supertile double-buffered tiles with
            # per-partition contiguous DMA rows.
            for s in range(N_SUP):
                y_s = y_first if s == 0 else y_dma(ypool, s)
                # g = 2m + r: frame n at phase a reads (m = n - 64 s + a//2, r = a%2)
                yv = y_s.rearrange("p e (m r) -> p e m r", r=2)
                sqs = []
                for m in range(M_GROUPS):
                    x_ps = psum1.tile([128, EX_PER_CORE, SUP], f32)
                    for a in range(4):
                        off = a // 2
                        rhs = yv[:, :, off : off + SUP, a % 2]
                        nc.tensor.matmul(
                            x_ps,
                            w_sb[:, a, m, :],
                            rhs,
                            start=(a == 0),
                            stop=(a == 3),
                        )
                    sq = ppool.tile([128, EX_PER_CORE, SUP], f32, tag=f"sq{m}")
                    nc.scalar.square(sq, x_ps)
                    sqs.append(sq)
                ps = []
                for m in range(K2_GROUPS):
                    p_t = ppool.tile([128, EX_PER_CORE, SUP], f32r, tag=f"p{m}")
                    nc.vector.tensor_add(p_t, sqs[m], sqs[m + K2_GROUPS])
                    ps.append(p_t)
                for c in range(4):
                    ac_ps = psum2.tile([128, LAG_COLS], f32)
                    for m in range(K2_GROUPS):
                        nc.tensor.matmul(
                            ac_ps,
                            ps[m][:, 2 * c : 2 * (c + 1), :],
                            c2_sb[:, m, :],
                            start=(m == 0),
                            stop=(m == K2_GROUPS - 1),
                        )
                    t = 4 * s + c
                    nc.vector.max(collect_v[:, t, :], ac_ps)
                    nc.vector.max_index(collect_i[:, t, :], collect_v[:, t, :], ac_ps)
                if s == 0:
                    cleanup_pass()

            half = N_TILES // 2
            nc.sync.dma_start(out=idx_out[:, :half], in_=collect_i[:, :half])
            nc.sync.dma_start(out=val_out[:, :half], in_=collect_v[:, :half])



            nc.sync.dma_start(out=idx_out[:, half:], in_=collect_i[:, half:])
            nc.sync.dma_start(out=val_out[:, half:], in_=collect_v[:, half:])
    nc.compile()
    return nc


def _get_nc():
    if "nc" not in _CACHE:
        _CACHE["nc"] = _build_nc()
        _CACHE["w"] = _weights()
    return _CACHE["nc"]


def modeled_exec_ns():
    """Per-core kernel time from the instruction cost model (TimelineSim).
    The axon client in this container has no NTFF profiling hook, so this
    is the best available device-time estimate."""
    from concourse import timeline_sim as ts

    class _Null:
        def __getattr__(self, name):
            return lambda *a, **k: None

    orig = ts._build_perfetto
    ts._build_perfetto = lambda core_id: _Null()
    try:
        return int(ts.TimelineSim(_get_nc(), trace=False).simulate())
    finally:
        ts._build_perfetto = orig


def _device_topk(xpad):
    """xpad: (64, T_PAD) fp32 -> (idx8, val8): (64, 641, 8) candidate lags/values."""
    nc = _get_nc()
    wh, c2h = _CACHE["w"]
    # block-transposed layout: xb[e, j, g] = xpad[e, 128 g + j]
    xb = np.ascontiguousarray(xpad.reshape(B, N_BLOCKS, 128).transpose(0, 2, 1))
    in_maps = []
    for r in range(N_CORES):
        in_maps.append(
            {
                "xb": np.ascontiguousarray(xb[r * EX_PER_CORE : (r + 1) * EX_PER_CORE]),
                "wdft": wh,
                "c2": c2h,
            }
        )
    trace = bool(int(__import__("os").environ.get("F0_TRACE", "0")))
    res = None
    for attempt in range(3):
        try:
            res = run_bass_kernel_spmd(nc, in_maps, list(range(N_CORES)), trace=trace)
            break
        except Exception:
            # transient NRT device errors have been observed; retry
            if attempt == 2:
                raise
    _CACHE["last_exec_time_ns"] = res.exec_time_ns
    idx8 = np.empty((B, N_FRAMES, 8), dtype=np.int64)
    val8 = np.empty((B, N_FRAMES, 8), dtype=np.float32)
    nmain = N_SUP * SUP
    for r in range(N_CORES):
        # device arrays [128 q, 40 t, 8]; q -> (e2, qq), t = 4s + c,
        # example e = 2c + e2, frame n = 64s + qq; frame 640 from idx_l/val_l
        di = res.results[r]["idx"].reshape(2, 64, N_SUP, 4, 8)
        dv = res.results[r]["val"].reshape(2, 64, N_SUP, 4, 8)
        sl = slice(r * EX_PER_CORE, (r + 1) * EX_PER_CORE)
        idx8[sl, :nmain] = (
            di.transpose(3, 0, 2, 1, 4).reshape(EX_PER_CORE, nmain, 8)
        )
        val8[sl, :nmain] = dv.transpose(3, 0, 2, 1, 4).reshape(EX_PER_CORE, nmain, 8)
        idx8[sl, nmain] = res.results[r]["idx_l"]
        val8[sl, nmain] = res.results[r]["val_l"]
    return idx8, val8


N_SLOTS = 4        # candidate lags rescored exactly per frame (of 8 returned)


def _exact_rescore(xpad, idx_slots):
    """Exact autocorrelation at the candidate lags: fp32 products (matching
    the reference's own fp32 product rounding scale), fp64 accumulation."""
    nb, nf, ns = idx_slots.shape
    starts = np.arange(nf) * HOP
    frames = np.lib.stride_tricks.sliding_window_view(xpad, FRAME_LEN, axis=1)[
        :, starts
    ]                                                     # (B, F, 512) fp32 view
    fpad = np.concatenate(
        [frames, np.zeros((nb, nf, FRAME_LEN), np.float32)], axis=2
    )                                                     # (B, F, 1024)
    lags = (idx_slots + MIN_PERIOD).astype(np.int32)      # (B, F, ns)
    i = np.arange(FRAME_LEN, dtype=np.int32)
    exact = np.empty(lags.shape, dtype=np.float64)
    for r in range(ns):
        shifted = np.take_along_axis(fpad, i + lags[:, :, r : r + 1], axis=2)
        exact[:, :, r] = (frames * shifted).sum(axis=2, dtype=np.float64)
    return exact


def _full_rescore(xpad, rows_b, rows_f):
    """All-224-lag exact autocorrelation argmax for ambiguous frames."""
    fr = np.stack(
        [xpad[b_, f_ * HOP : f_ * HOP + FRAME_LEN] for b_, f_ in zip(rows_b, rows_f)]
    ).astype(np.float64)                                  # (R, 512)
    ac = np.empty((len(rows_b), N_LAGS))
    for j, p in enumerate(range(MIN_PERIOD, 256)):
        ac[:, j] = np.einsum("ri,ri->r", fr[:, : FRAME_LEN - p], fr[:, p:])
    return np.argmax(ac, axis=1).astype(np.int64)


def kernel(waveform):
    waveform = np.asarray(waveform, dtype=np.float32)
    x = waveform[:, 0, :]
    xpad = np.pad(x, ((0, 0), (PAD, PAD)), mode="reflect")
    idx8, val8 = _device_topk(xpad)

    idx4 = idx8[:, :, :N_SLOTS]
    exact = _exact_rescore(xpad, idx4)
    # among the candidates pick the exact-max; ties -> smallest lag
    order = np.argsort(idx4, axis=2)                       # evaluate in lag order
    exact_sorted = np.take_along_axis(exact, order, axis=2)
    idx_sorted = np.take_along_axis(idx4, order, axis=2)
    best_slot = np.argmax(exact_sorted, axis=2)            # first max in lag order
    best_idx = np.take_along_axis(idx_sorted, best_slot[..., None], axis=2)[..., 0]

    # Frames where the approximate top-4 window may not contain the true
    # argmax: approximate spread below 10x the measured f32r error bound
    # (end-to-end |approx/N - exact| <= 4.9e-4 * top1 scale on this
    # distribution) -> exact argmax over all 224 lags instead.
    scale = np.abs(val8[:, :, 0]) + 1e-20
    spread = val8[:, :, 0] - val8[:, :, N_SLOTS - 1]
    risky = spread < 5e-3 * scale
    if np.any(risky):
        rb, rf = np.nonzero(risky)
        best_idx[rb, rf] = _full_rescore(xpad, rb, rf)

    period = best_idx.astype(np.float32) + np.float32(MIN_PERIOD)
    f0 = np.float32(SR) / (period + np.float32(1e-8))
    return np.clip(f0, np.float32(50.0), np.float32(500.0)).astype(np.float32)


# revision 41
# speedup vs baseline: 1.0844x; 1.0174x over previous
"""F0 extractor kernel for trn2 (8 NeuronCores, batch-data-parallel).

Math: for each length-512 frame (hop 256) of the reflect-padded waveform,
f0 = SR / argmax_{p in [32,256)} autocorr(frame, p).  The L2 normalization
in the reference divides every lag of a frame by the same positive scalar,
so it cannot change the argmax and is skipped.

Device pipeline (per core, 8 examples), via autocorr = IDFT(|DFT|^2):
  1. Host pre-transposes the padded signal into 128-sample-block layout
     xb[e, j, g] = xpad[e, 128 g + j] so every DMA row is contiguous;
     per-supertile (64 frames/example) double-buffered SBUF tiles.  The
     four contraction K-tiles of each frame are strided views (frames
     overlap 50%, blocks are stored once).
  2. Forward DFT-767 of every frame as float32r matmuls (1 cycle/row)
     with shared trig weights: X[row, frame] in PSUM; 768 rows = 384 cos
     + 384 sin bins (N odd -> no Nyquist special case).
  3. ScalarE Square into SBUF, VectorE adds Re^2+Im^2 (rows k and 384+k
     are partition-aligned) -> P[bin, frame], 384 rows.
  4. Inverse transform as matmuls: ac[frame, lag] = sum_bin P * C2 with
     P slices stationary so frames land on partitions.  Lag columns
     padded 224->256 (full-rate f32r needs N>=256) with -sum w_k P_k,
     a provable lower bound of every true lag, so pads never win.
  5. VectorE max / max_index straight off PSUM: top-8 values + indices
     per frame -> DRAM.

float32r is TF32-ish: measured end-to-end |approx/N - exact| <= 4.9e-4
of the top-1 scale on this distribution, and the exact argmax always sits
in approx slots 0-1.  The host exactly rescores the top-4 candidate lags
of every frame (fp32 products, fp64 accumulation) and falls back to all
224 lags when the top-4 spread is within 5e-3 of the scale.  Exact-vs-
reference ordering is safe: the top-2 relative gap exceeds 1e-5 on every
frame of this distribution (fp32 reference noise is ~1e-6).
"""

import numpy as np

import concourse.bacc as bacc
import concourse.bass as bass
import concourse.tile as tile
from concourse import mybir
from concourse.bass_utils import run_bass_kernel_spmd

SR = 16000
HOP = 256
FRAME_LEN = 512
PAD = 256
MIN_PERIOD = 32
N_LAGS = 224          # lags 32..255
LAG_COLS = 256        # padded lag columns for full-rate f32r matmul
B = 64
T = 163840
N_FRAMES = 641
N_CORES = 8
EX_PER_CORE = B // N_CORES
T_PAD = T + 2 * PAD            # 164352 = 642 * 256
N_DFT = 767                    # odd: bins 0..383, no Nyquist special case
N_BINS = 384                   # real bins 0..383
ROWS = 768                     # 384 cos rows then 384 sin rows (sin_0 = 0 row)
M_GROUPS = 6                   # 768 / 128 forward output groups
K2_GROUPS = 3                  # 384 power rows / 128 for the inverse matmul
SUP = 64                       # frames per example per supertile
N_SUP = 10                     # frames 0..639; frame 640 via a cleanup pass
N_TILES = N_SUP * 4            # 40 tiles of 128 frames per core

f32 = mybir.dt.float32
f32r = mybir.dt.float32r
u32 = mybir.dt.uint32

_CACHE = {}


def _weights():
    i = np.arange(FRAME_LEN, dtype=np.float64)
    k = np.arange(N_BINS, dtype=np.float64)
    ang = 2.0 * np.pi * np.outer(i, k) / N_DFT            # [512, 384]
    w_fwd = np.concatenate([np.cos(ang), np.sin(ang)], axis=1)            # [512,768]
    # host layout [j, a, m, mb]: i = 128a + j, row = 128m + mb
    wh = (
        w_fwd.reshape(4, 128, M_GROUPS, 128)
        .transpose(1, 0, 2, 3)
        .astype(np.float32)
    )
    wk = np.where(k == 0, 1.0, 2.0)
    p = np.arange(MIN_PERIOD, MIN_PERIOD + N_LAGS, dtype=np.float64)
    c2 = wk[:, None] * np.cos(2.0 * np.pi * np.outer(k, p) / N_DFT)       # [384,224]
    pad = np.repeat(-wk[:, None], LAG_COLS - N_LAGS, axis=1)              # [384,32]
    c2 = np.concatenate([c2, pad], axis=1)                                # [384,256]
    c2h = c2.reshape(K2_GROUPS, 128, LAG_COLS).transpose(1, 0, 2).astype(np.float32)
    return wh, c2h


N_BLOCKS = T_PAD // 128          # 1284 128-blocks per example (no padding)
G_COLS = N_BLOCKS


def _build_nc():
    nc = bacc.Bacc("TRN2", target_bir_lowering=False, debug=False, num_devices=1)
    x = nc.dram_tensor("xb", [EX_PER_CORE, 128, G_COLS], f32r, kind="ExternalInput").ap()
    wdft = nc.dram_tensor("wdft", [128, 4, M_GROUPS, 128], f32r, kind="ExternalInput").ap()
    c2 = nc.dram_tensor("c2", [128, K2_GROUPS, LAG_COLS], f32r, kind="ExternalInput").ap()
    idx_out = nc.dram_tensor("idx", [128, N_TILES, 8], u32, kind="ExternalOutput").ap()
    val_out = nc.dram_tensor("val", [128, N_TILES, 8], f32, kind="ExternalOutput").ap()
    idx_l = nc.dram_tensor("idx_l", [EX_PER_CORE, 8], u32, kind="ExternalOutput").ap()
    val_l = nc.dram_tensor("val_l", [EX_PER_CORE, 8], f32, kind="ExternalOutput").ap()

    with tile.TileContext(nc) as tc:
        with (
            tc.tile_pool(name="singles", bufs=1) as singles,
            tc.tile_pool(name="ypool", bufs=3) as ypool,
            tc.tile_pool(name="ppool", bufs=3) as ppool,
            tc.tile_pool(name="psum1", bufs=5, space="PSUM") as psum1,
            tc.tile_pool(name="psum2", bufs=3, space="PSUM") as psum2,
        ):
            # DMA issue order = first-use order: supertile-0 signal, then the
            # six forward-weight chunks, then the inverse weights.
            GS = 2 * SUP + 2          # 130 block columns per supertile

            def y_dma(pool, s):
                y_s = pool.tile([128, EX_PER_CORE, GS], f32r, tag="ys")
                src = bass.AP(
                    tensor=x.tensor,
                    offset=128 * s,
                    ap=[[G_COLS, 128], [128 * G_COLS, EX_PER_CORE], [1, GS]],
                )
                nc.sync.dma_start(out=y_s, in_=src)
                return y_s

            w_sb = singles.tile([128, 4, M_GROUPS, 128], f32r, tag="w")
            c2_sb = singles.tile([128, K2_GROUPS, LAG_COLS], f32r, tag="c2")
            # the very first matmul needs only W[a=0, m=0]: ship that 64 KB
            # slice first, then supertile-0's signal, then the rest
            nc.sync.dma_start(out=w_sb[:, 0, 0, :], in_=wdft[:, 0, 0, :])
            y_first = y_dma(ypool, 0)
            for a in range(1, 4):
                nc.sync.dma_start(out=w_sb[:, a, 0, :], in_=wdft[:, a, 0, :])
            for m in range(1, M_GROUPS):
                nc.sync.dma_start(out=w_sb[:, :, m, :], in_=wdft[:, :, m, :])
            nc.sync.dma_start(out=c2_sb, in_=c2)


            collect_i = singles.tile([128, N_TILES, 8], u32, tag="ci")
            collect_v = singles.tile([128, N_TILES, 8], f32, tag="cv")

            def cleanup_pass():
                # cleanup pass: frame 640 of each example (blocks 1280..1283)
                y_l = singles.tile([128, EX_PER_CORE, 4], f32r, tag="yl")
                src = bass.AP(
                    tensor=x.tensor,
                    offset=2 * N_SUP * SUP,
                    ap=[[G_COLS, 128], [128 * G_COLS, EX_PER_CORE], [1, 4]],
                )
                nc.sync.dma_start(out=y_l, in_=src)
                yvl = y_l.rearrange("p e (m r) -> p e m r", r=2)
                sqs = []
                for m in range(M_GROUPS):
                    x_ps = psum1.tile([128, EX_PER_CORE], f32)
                    for a in range(4):
                        rhs = yvl[:, :, a // 2, a % 2]
                        nc.tensor.matmul(
                            x_ps, w_sb[:, a, m, :], rhs, start=(a == 0), stop=(a == 3)
                        )
                    sq = ppool.tile([128, EX_PER_CORE], f32, tag=f"sql{m}")
                    nc.scalar.square(sq, x_ps)
                    sqs.append(sq)
                ps = []
                for m in range(K2_GROUPS):
                    p_t = ppool.tile([128, EX_PER_CORE], f32r, tag=f"pl{m}")
                    nc.vector.tensor_add(p_t, sqs[m], sqs[m + K2_GROUPS])
                    ps.append(p_t)
                ac_ps = psum2.tile([EX_PER_CORE, LAG_COLS], f32)
                for m in range(K2_GROUPS):
                    nc.tensor.matmul(
                        ac_ps, ps[m], c2_sb[:, m, :],
                        start=(m == 0), stop=(m == K2_GROUPS - 1),
                    )
                vl = singles.tile([EX_PER_CORE, 8], f32, tag="vl")
                il = singles.tile([EX_PER_CORE, 8], u32, tag="il")
                nc.vector.max(vl, ac_ps)
                nc.vector.max_index(il, vl, ac_ps)
                nc.sync.dma_start(out=val_l, in_=vl)
                nc.sync.dma_start(out=idx_l, in_=il)

            # Signal in block layout (host pre-transposed): xb[e, j, g] =
            # xpad[e, 128g + j]; per-supertile double-buffered tiles with
            # per-partition contiguous DMA rows.
            for s in range(N_SUP):
                y_s = y_first if s == 0 else y_dma(ypool, s)
                # g = 2m + r: frame n at phase a reads (m = n - 64 s + a//2, r = a%2)
                yv = y_s.rearrange("p e (m r) -> p e m r", r=2)
                sqs = []
                for m in range(M_GROUPS):
                    x_ps = psum1.tile([128, EX_PER_CORE, SUP], f32)
                    for a in range(4):
                        off = a // 2
                        rhs = yv[:, :, off : off + SUP, a % 2]
                        nc.tensor.matmul(
                            x_ps,
                            w_sb[:, a, m, :],
                            rhs,
                            start=(a == 0),
                            stop=(a == 3),
                        )
                    sq = ppool.tile([128, EX_PER_CORE, SUP], f32, tag=f"sq{m}")
                    nc.scalar.square(sq, x_ps)
                    sqs.append(sq)
                ps = []
                for m in range(K2_GROUPS):
                    p_t = ppool.tile([128, EX_PER_CORE, SUP], f32r, tag=f"p{m}")
                    nc.vector.tensor_add(p_t, sqs[m], sqs[m + K2_GROUPS])
                    ps.append(p_t)
                for c in range(4):
                    ac_ps = psum2.tile([128, LAG_COLS], f32)
                    for m in range(K2_GROUPS):
                        nc.tensor.matmul(
                            ac_ps,
                            ps[m][:, 2 * c : 2 * (c + 1), :],
                            c2_sb[:, m, :],
                            start=(m == 0),
                            stop=(m == K2_GROUPS - 1),
                        )
                    t = 4 * s + c
                    nc.vector.max(collect_v[:, t, :], ac_ps)
                    nc.vector.max_index(collect_i[:, t, :], collect_v[:, t, :], ac_ps)
                if s == 0:
                    cleanup_pass()

            q = N_TILES // 4
            for qi in range(4):
                sl = slice(qi * q, (qi + 1) * q)
                nc.sync.dma_start(out=idx_out[:, sl], in_=collect_i[:, sl])
                nc.sync.dma_start(out=val_out[:, sl], in_=collect_v[:, sl])
    nc.compile()
    return nc


def _get_nc():
    if "nc" not in _CACHE:
        _CACHE["nc"] = _build_nc()
        _CACHE["w"] = _weights()
    return _CACHE["nc"]


def modeled_exec_ns():
    """Per-core kernel time from the instruction cost model (TimelineSim).
    The axon client in this container has no NTFF profiling hook, so this
    is the best available device-time estimate."""
    from concourse import timeline_sim as ts

    class _Null:
        def __getattr__(self, name):
            return lambda *a, **k: None

    orig = ts._build_perfetto
    ts._build_perfetto = lambda core_id: _Null()
    try:
        return int(ts.TimelineSim(_get_nc(), trace=False).simulate())
    finally:
        ts._build_perfetto = orig


def _device_topk(xpad):
    """xpad: (64, T_PAD) fp32 -> (idx8, val8): (64, 641, 8) candidate lags/values."""
    nc = _get_nc()
    wh, c2h = _CACHE["w"]
    # block-transposed layout: xb[e, j, g] = xpad[e, 128 g + j]
    xb = np.ascontiguousarray(xpad.reshape(B, N_BLOCKS, 128).transpose(0, 2, 1))
    in_maps = []
    for r in range(N_CORES):
        in_maps.append(
            {
                "xb": np.ascontiguousarray(xb[r * EX_PER_CORE : (r + 1) * EX_PER_CORE]),
                "wdft": wh,
                "c2": c2h,
            }
        )
    trace = bool(int(__import__("os").environ.get("F0_TRACE", "0")))
    res = None
    for attempt in range(3):
        try:
            res = run_bass_kernel_spmd(nc, in_maps, list(range(N_CORES)), trace=trace)
            break
        except Exception:
            # transient NRT device errors have been observed; retry
            if attempt == 2:
                raise
    _CACHE["last_exec_time_ns"] = res.exec_time_ns
    idx8 = np.empty((B, N_FRAMES, 8), dtype=np.int64)
    val8 = np.empty((B, N_FRAMES, 8), dtype=np.float32)
    nmain = N_SUP * SUP
    for r in range(N_CORES):
        # device arrays [128 q, 40 t, 8]; q -> (e2, qq), t = 4s + c,
        # example e = 2c + e2, frame n = 64s + qq; frame 640 from idx_l/val_l
        di = res.results[r]["idx"].reshape(2, 64, N_SUP, 4, 8)
        dv = res.results[r]["val"].reshape(2, 64, N_SUP, 4, 8)
        sl = slice(r * EX_PER_CORE, (r + 1) * EX_PER_CORE)
        idx8[sl, :nmain] = (
            di.transpose(3, 0, 2, 1, 4).reshape(EX_PER_CORE, nmain, 8)
        )
        val8[sl, :nmain] = dv.transpose(3, 0, 2, 1, 4).reshape(EX_PER_CORE, nmain, 8)
        idx8[sl, nmain] = res.results[r]["idx_l"]
        val8[sl, nmain] = res.results[r]["val_l"]
    return idx8, val8


N_SLOTS = 4        # candidate lags rescored exactly per frame (of 8 returned)


def _exact_rescore(xpad, idx_slots):
    """Exact autocorrelation at the candidate lags: fp32 products (matching
    the reference's own fp32 product rounding scale), fp64 accumulation."""
    nb, nf, ns = idx_slots.shape
    starts = np.arange(nf) * HOP
    frames = np.lib.stride_tricks.sliding_window_view(xpad, FRAME_LEN, axis=1)[
        :, starts
    ]                                                     # (B, F, 512) fp32 view
    fpad = np.concatenate(
        [frames, np.zeros((nb, nf, FRAME_LEN), np.float32)], axis=2
    )                                                     # (B, F, 1024)
    lags = (idx_slots + MIN_PERIOD).astype(np.int32)      # (B, F, ns)
    i = np.arange(FRAME_LEN, dtype=np.int32)
    exact = np.empty(lags.shape, dtype=np.float64)
    for r in range(ns):
        shifted = np.take_along_axis(fpad, i + lags[:, :, r : r + 1], axis=2)
        exact[:, :, r] = (frames * shifted).sum(axis=2, dtype=np.float64)
    return exact


def _full_rescore(xpad, rows_b, rows_f):
    """All-224-lag exact autocorrelation argmax for ambiguous frames."""
    fr = np.stack(
        [xpad[b_, f_ * HOP : f_ * HOP + FRAME_LEN] for b_, f_ in zip(rows_b, rows_f)]
    ).astype(np.float64)                                  # (R, 512)
    ac = np.empty((len(rows_b), N_LAGS))
    for j, p in enumerate(range(MIN_PERIOD, 256)):
        ac[:, j] = np.einsum("ri,ri->r", fr[:, : FRAME_LEN - p], fr[:, p:])
    return np.argmax(ac, axis=1).astype(np.int64)


def kernel(waveform):
    waveform = np.asarray(waveform, dtype=np.float32)
    x = waveform[:, 0, :]
    xpad = np.pad(x, ((0, 0), (PAD, PAD)), mode="reflect")
    idx8, val8 = _device_topk(xpad)

    idx4 = idx8[:, :, :N_SLOTS]
    exact = _exact_rescore(xpad, idx4)
    # among the candidates pick the exact-max; ties -> smallest lag
    order = np.argsort(idx4, axis=2)                       # evaluate in lag order
    exact_sorted = np.take_along_axis(exact, order, axis=2)
    idx_sorted = np.take_along_axis(idx4, order, axis=2)
    best_slot = np.argmax(exact_sorted, axis=2)            # first max in lag order
    best_idx = np.take_along_axis(idx_sorted, best_slot[..., None], axis=2)[..., 0]

    # Frames where the approximate top-4 window may not contain the true
    # argmax: approximate spread below 10x the measured f32r error bound
    # (end-to-end |approx/N - exact| <= 4.9e-4 * top1 scale on this
    # distribution) -> exact argmax over all 224 lags instead.
    scale = np.abs(val8[:, :, 0]) + 1e-20
    spread = val8[:, :, 0] - val8[:, :, N_SLOTS - 1]
    risky = spread < 5e-3 * scale
    if np.any(risky):
        rb, rf = np.nonzero(risky)
        best_idx[rb, rf] = _full_rescore(xpad, rb, rf)

    period = best_idx.astype(np.float32) + np.float32(MIN_PERIOD)
    f0 = np.float32(SR) / (period + np.float32(1e-8))
    return np.clip(f0, np.float32(50.0), np.float32(500.0)).astype(np.float32)


# revision 42
# speedup vs baseline: 1.0929x; 1.0079x over previous
"""F0 extractor kernel for trn2 (8 NeuronCores, batch-data-parallel).

Math: for each length-512 frame (hop 256) of the reflect-padded waveform,
f0 = SR / argmax_{p in [32,256)} autocorr(frame, p).  The L2 normalization
in the reference divides every lag of a frame by the same positive scalar,
so it cannot change the argmax and is skipped.

Device pipeline (per core, 8 examples), via autocorr = IDFT(|DFT|^2):
  1. Host pre-transposes the padded signal into 128-sample-block layout
     xb[e, j, g] = xpad[e, 128 g + j] so every DMA row is contiguous;
     per-supertile (64 frames/example) double-buffered SBUF tiles.  The
     four contraction K-tiles of each frame are strided views (frames
     overlap 50%, blocks are stored once).
  2. Forward DFT-767 of every frame as float32r matmuls (1 cycle/row)
     with shared trig weights: X[row, frame] in PSUM; 768 rows = 384 cos
     + 384 sin bins (N odd -> no Nyquist special case).
  3. ScalarE Square into SBUF, VectorE adds Re^2+Im^2 (rows k and 384+k
     are partition-aligned) -> P[bin, frame], 384 rows.
  4. Inverse transform as matmuls: ac[frame, lag] = sum_bin P * C2 with
     P slices stationary so frames land on partitions.  Lag columns
     padded 224->256 (full-rate f32r needs N>=256) with -sum w_k P_k,
     a provable lower bound of every true lag, so pads never win.
  5. VectorE max / max_index straight off PSUM: top-8 values + indices
     per frame -> DRAM.

float32r is TF32-ish: measured end-to-end |approx/N - exact| <= 4.9e-4
of the top-1 scale on this distribution, and the exact argmax always sits
in approx slots 0-1.  The host exactly rescores the top-4 candidate lags
of every frame (fp32 products, fp64 accumulation) and falls back to all
224 lags when the top-4 spread is within 5e-3 of the scale.  Exact-vs-
reference ordering is safe: the top-2 relative gap exceeds 1e-5 on every
frame of this distribution (fp32 reference noise is ~1e-6).
"""

import numpy as np

import concourse.bacc as bacc
import concourse.bass as bass
import concourse.tile as tile
from concourse import mybir
from concourse.bass_utils import run_bass_kernel_spmd

SR = 16000
HOP = 256
FRAME_LEN = 512
PAD = 256
MIN_PERIOD = 32
N_LAGS = 224          # lags 32..255
LAG_COLS = 256        # padded lag columns for full-rate f32r matmul
B = 64
T = 163840
N_FRAMES = 641
N_CORES = 8
EX_PER_CORE = B // N_CORES
T_PAD = T + 2 * PAD            # 164352 = 642 * 256
N_DFT = 767                    # odd: bins 0..383, no Nyquist special case
N_BINS = 384                   # real bins 0..383
ROWS = 768                     # 384 cos rows then 384 sin rows (sin_0 = 0 row)
M_GROUPS = 6                   # 768 / 128 forward output groups
K2_GROUPS = 3                  # 384 power rows / 128 for the inverse matmul
SUP = 64                       # frames per example per supertile
N_SUP = 10                     # frames 0..639; frame 640 via a cleanup pass
N_TILES = N_SUP * 4            # 40 tiles of 128 frames per core

f32 = mybir.dt.float32
f32r = mybir.dt.float32r
u32 = mybir.dt.uint32

_CACHE = {}


def _weights():
    i = np.arange(FRAME_LEN, dtype=np.float64)
    k = np.arange(N_BINS, dtype=np.float64)
    ang = 2.0 * np.pi * np.outer(i, k) / N_DFT            # [512, 384]
    w_fwd = np.concatenate([np.cos(ang), np.sin(ang)], axis=1)            # [512,768]
    # host layout [j, a, m, mb]: i = 128a + j, row = 128m + mb
    wh = (
        w_fwd.reshape(4, 128, M_GROUPS, 128)
        .transpose(1, 0, 2, 3)
        .astype(np.float32)
    )
    wk = np.where(k == 0, 1.0, 2.0)
    p = np.arange(MIN_PERIOD, MIN_PERIOD + N_LAGS, dtype=np.float64)
    c2 = wk[:, None] * np.cos(2.0 * np.pi * np.outer(k, p) / N_DFT)       # [384,224]
    pad = np.repeat(-wk[:, None], LAG_COLS - N_LAGS, axis=1)              # [384,32]
    c2 = np.concatenate([c2, pad], axis=1)                                # [384,256]
    c2h = c2.reshape(K2_GROUPS, 128, LAG_COLS).transpose(1, 0, 2).astype(np.float32)
    return wh, c2h


N_BLOCKS = T_PAD // 128          # 1284 128-blocks per example (no padding)
G_COLS = N_BLOCKS


def _build_nc():
    nc = bacc.Bacc("TRN2", target_bir_lowering=False, debug=False, num_devices=1)
    x = nc.dram_tensor("xb", [EX_PER_CORE, 128, G_COLS], f32r, kind="ExternalInput").ap()
    wdft = nc.dram_tensor("wdft", [128, 4, M_GROUPS, 128], f32r, kind="ExternalInput").ap()
    c2 = nc.dram_tensor("c2", [128, K2_GROUPS, LAG_COLS], f32r, kind="ExternalInput").ap()
    idx_out = nc.dram_tensor("idx", [128, N_TILES, 8], u32, kind="ExternalOutput").ap()
    val_out = nc.dram_tensor("val", [128, N_TILES, 8], f32, kind="ExternalOutput").ap()
    idx_l = nc.dram_tensor("idx_l", [EX_PER_CORE, 8], u32, kind="ExternalOutput").ap()
    val_l = nc.dram_tensor("val_l", [EX_PER_CORE, 8], f32, kind="ExternalOutput").ap()

    with tile.TileContext(nc) as tc:
        with (
            tc.tile_pool(name="singles", bufs=1) as singles,
            tc.tile_pool(name="ypool", bufs=3) as ypool,
            tc.tile_pool(name="ppool", bufs=3) as ppool,
            tc.tile_pool(name="psum1", bufs=4, space="PSUM") as psum1,
            tc.tile_pool(name="psum2", bufs=4, space="PSUM") as psum2,
        ):
            # DMA issue order = first-use order: supertile-0 signal, then the
            # six forward-weight chunks, then the inverse weights.
            GS = 2 * SUP + 2          # 130 block columns per supertile

            def y_dma(pool, s):
                y_s = pool.tile([128, EX_PER_CORE, GS], f32r, tag="ys")
                src = bass.AP(
                    tensor=x.tensor,
                    offset=128 * s,
                    ap=[[G_COLS, 128], [128 * G_COLS, EX_PER_CORE], [1, GS]],
                )
                nc.sync.dma_start(out=y_s, in_=src)
                return y_s

            w_sb = singles.tile([128, 4, M_GROUPS, 128], f32r, tag="w")
            c2_sb = singles.tile([128, K2_GROUPS, LAG_COLS], f32r, tag="c2")
            # the very first matmul needs only W[a=0, m=0]: ship that 64 KB
            # slice first, then supertile-0's signal, then the rest
            nc.sync.dma_start(out=w_sb[:, 0, 0, :], in_=wdft[:, 0, 0, :])
            y_first = y_dma(ypool, 0)
            for a in range(1, 4):
                nc.sync.dma_start(out=w_sb[:, a, 0, :], in_=wdft[:, a, 0, :])
            for m in range(1, M_GROUPS):
                nc.sync.dma_start(out=w_sb[:, :, m, :], in_=wdft[:, :, m, :])
            nc.sync.dma_start(out=c2_sb, in_=c2)


            collect_i = singles.tile([128, N_TILES, 8], u32, tag="ci")
            collect_v = singles.tile([128, N_TILES, 8], f32, tag="cv")

            def cleanup_pass():
                # cleanup pass: frame 640 of each example (blocks 1280..1283)
                y_l = singles.tile([128, EX_PER_CORE, 4], f32r, tag="yl")
                src = bass.AP(
                    tensor=x.tensor,
                    offset=2 * N_SUP * SUP,
                    ap=[[G_COLS, 128], [128 * G_COLS, EX_PER_CORE], [1, 4]],
                )
                nc.sync.dma_start(out=y_l, in_=src)
                yvl = y_l.rearrange("p e (m r) -> p e m r", r=2)
                sqs = []
                for m in range(M_GROUPS):
                    x_ps = psum1.tile([128, EX_PER_CORE], f32)
                    for a in range(4):
                        rhs = yvl[:, :, a // 2, a % 2]
                        nc.tensor.matmul(
                            x_ps, w_sb[:, a, m, :], rhs, start=(a == 0), stop=(a == 3)
                        )
                    sq = ppool.tile([128, EX_PER_CORE], f32, tag=f"sql{m}")
                    nc.scalar.square(sq, x_ps)
                    sqs.append(sq)
                ps = []
                for m in range(K2_GROUPS):
                    p_t = ppool.tile([128, EX_PER_CORE], f32r, tag=f"pl{m}")
                    nc.vector.tensor_add(p_t, sqs[m], sqs[m + K2_GROUPS])
                    ps.append(p_t)
                ac_ps = psum2.tile([EX_PER_CORE, LAG_COLS], f32)
                for m in range(K2_GROUPS):
                    nc.tensor.matmul(
                        ac_ps, ps[m], c2_sb[:, m, :],
                        start=(m == 0), stop=(m == K2_GROUPS - 1),
                    )
                vl = singles.tile([EX_PER_CORE, 8], f32, tag="vl")
                il = singles.tile([EX_PER_CORE, 8], u32, tag="il")
                nc.vector.max(vl, ac_ps)
                nc.vector.max_index(il, vl, ac_ps)
                nc.sync.dma_start(out=val_l, in_=vl)
                nc.sync.dma_start(out=idx_l, in_=il)

            # Signal in block layout (host pre-transposed): xb[e, j, g] =
            # xpad[e, 128g + j]; per-supertile double-buffered tiles with
            # per-partition contiguous DMA rows.
            for s in range(N_SUP):
                y_s = y_first if s == 0 else y_dma(ypool, s)
                # g = 2m + r: frame n at phase a reads (m = n - 64 s + a//2, r = a%2)
                yv = y_s.rearrange("p e (m r) -> p e m r", r=2)
                sqs = []
                for m in range(M_GROUPS):
                    x_ps = psum1.tile([128, EX_PER_CORE, SUP], f32)
                    for a in range(4):
                        off = a // 2
                        rhs = yv[:, :, off : off + SUP, a % 2]
                        nc.tensor.matmul(
                            x_ps,
                            w_sb[:, a, m, :],
                            rhs,
                            start=(a == 0),
                            stop=(a == 3),
                        )
                    sq = ppool.tile([128, EX_PER_CORE, SUP], f32, tag=f"sq{m}")
                    nc.scalar.square(sq, x_ps)
                    sqs.append(sq)
                ps = []
                for m in range(K2_GROUPS):
                    p_t = ppool.tile([128, EX_PER_CORE, SUP], f32r, tag=f"p{m}")
                    nc.vector.tensor_add(p_t, sqs[m], sqs[m + K2_GROUPS])
                    ps.append(p_t)
                for c in range(4):
                    ac_ps = psum2.tile([128, LAG_COLS], f32)
                    for m in range(K2_GROUPS):
                        nc.tensor.matmul(
                            ac_ps,
                            ps[m][:, 2 * c : 2 * (c + 1), :],
                            c2_sb[:, m, :],
                            start=(m == 0),
                            stop=(m == K2_GROUPS - 1),
                        )
                    t = 4 * s + c
                    nc.vector.max(collect_v[:, t, :], ac_ps)
                    nc.vector.max_index(collect_i[:, t, :], collect_v[:, t, :], ac_ps)
                if s == 0:
                    cleanup_pass()

            q = N_TILES // 4
            for qi in range(4):
                sl = slice(qi * q, (qi + 1) * q)
                nc.sync.dma_start(out=idx_out[:, sl], in_=collect_i[:, sl])
                nc.sync.dma_start(out=val_out[:, sl], in_=collect_v[:, sl])
    nc.compile()
    return nc


def _get_nc():
    if "nc" not in _CACHE:
        _CACHE["nc"] = _build_nc()
        _CACHE["w"] = _weights()
    return _CACHE["nc"]


def modeled_exec_ns():
    """Per-core kernel time from the instruction cost model (TimelineSim).
    The axon client in this container has no NTFF profiling hook, so this
    is the best available device-time estimate."""
    from concourse import timeline_sim as ts

    class _Null:
        def __getattr__(self, name):
            return lambda *a, **k: None

    orig = ts._build_perfetto
    ts._build_perfetto = lambda core_id: _Null()
    try:
        return int(ts.TimelineSim(_get_nc(), trace=False).simulate())
    finally:
        ts._build_perfetto = orig


def _device_topk(xpad):
    """xpad: (64, T_PAD) fp32 -> (idx8, val8): (64, 641, 8) candidate lags/values."""
    nc = _get_nc()
    wh, c2h = _CACHE["w"]
    # block-transposed layout: xb[e, j, g] = xpad[e, 128 g + j]
    xb = np.ascontiguousarray(xpad.reshape(B, N_BLOCKS, 128).transpose(0, 2, 1))
    in_maps = []
    for r in range(N_CORES):
        in_maps.append(
            {
                "xb": np.ascontiguousarray(xb[r * EX_PER_CORE : (r + 1) * EX_PER_CORE]),
                "wdft": wh,
                "c2": c2h,
            }
        )
    trace = bool(int(__import__("os").environ.get("F0_TRACE", "0")))
    res = None
    for attempt in range(3):
        try:
            res = run_bass_kernel_spmd(nc, in_maps, list(range(N_CORES)), trace=trace)
            break
        except Exception:
            # transient NRT device errors have been observed; retry
            if attempt == 2:
                raise
    _CACHE["last_exec_time_ns"] = res.exec_time_ns
    idx8 = np.empty((B, N_FRAMES, 8), dtype=np.int64)
    val8 = np.empty((B, N_FRAMES, 8), dtype=np.float32)
    nmain = N_SUP * SUP
    for r in range(N_CORES):
        # device arrays [128 q, 40 t, 8]; q -> (e2, qq), t = 4s + c,
        # example e = 2c + e2, frame n = 64s + qq; frame 640 from idx_l/val_l
        di = res.results[r]["idx"].reshape(2, 64, N_SUP, 4, 8)
        dv = res.results[r]["val"].reshape(2, 64, N_SUP, 4, 8)
        sl = slice(r * EX_PER_CORE, (r + 1) * EX_PER_CORE)
        idx8[sl, :nmain] = (
            di.transpose(3, 0, 2, 1, 4).reshape(EX_PER_CORE, nmain, 8)
        )
        val8[sl, :nmain] = dv.transpose(3, 0, 2, 1, 4).reshape(EX_PER_CORE, nmain, 8)
        idx8[sl, nmain] = res.results[r]["idx_l"]
        val8[sl, nmain] = res.results[r]["val_l"]
    return idx8, val8


N_SLOTS = 4        # candidate lags rescored exactly per frame (of 8 returned)


def _exact_rescore(xpad, idx_slots):
    """Exact autocorrelation at the candidate lags: fp32 products (matching
    the reference's own fp32 product rounding scale), fp64 accumulation."""
    nb, nf, ns = idx_slots.shape
    starts = np.arange(nf) * HOP
    frames = np.lib.stride_tricks.sliding_window_view(xpad, FRAME_LEN, axis=1)[
        :, starts
    ]                                                     # (B, F, 512) fp32 view
    fpad = np.concatenate(
        [frames, np.zeros((nb, nf, FRAME_LEN), np.float32)], axis=2
    )                                                     # (B, F, 1024)
    lags = (idx_slots + MIN_PERIOD).astype(np.int32)      # (B, F, ns)
    i = np.arange(FRAME_LEN, dtype=np.int32)
    exact = np.empty(lags.shape, dtype=np.float64)
    for r in range(ns):
        shifted = np.take_along_axis(fpad, i + lags[:, :, r : r + 1], axis=2)
        exact[:, :, r] = (frames * shifted).sum(axis=2, dtype=np.float64)
    return exact


def _full_rescore(xpad, rows_b, rows_f):
    """All-224-lag exact autocorrelation argmax for ambiguous frames."""
    fr = np.stack(
        [xpad[b_, f_ * HOP : f_ * HOP + FRAME_LEN] for b_, f_ in zip(rows_b, rows_f)]
    ).astype(np.float64)                                  # (R, 512)
    ac = np.empty((len(rows_b), N_LAGS))
    for j, p in enumerate(range(MIN_PERIOD, 256)):
        ac[:, j] = np.einsum("ri,ri->r", fr[:, : FRAME_LEN - p], fr[:, p:])
    return np.argmax(ac, axis=1).astype(np.int64)


def kernel(waveform):
    waveform = np.asarray(waveform, dtype=np.float32)
    x = waveform[:, 0, :]
    xpad = np.pad(x, ((0, 0), (PAD, PAD)), mode="reflect")
    idx8, val8 = _device_topk(xpad)

    idx4 = idx8[:, :, :N_SLOTS]
    exact = _exact_rescore(xpad, idx4)
    # among the candidates pick the exact-max; ties -> smallest lag
    order = np.argsort(idx4, axis=2)                       # evaluate in lag order
    exact_sorted = np.take_along_axis(exact, order, axis=2)
    idx_sorted = np.take_along_axis(idx4, order, axis=2)
    best_slot = np.argmax(exact_sorted, axis=2)            # first max in lag order
    best_idx = np.take_along_axis(idx_sorted, best_slot[..., None], axis=2)[..., 0]

    # Frames where the approximate top-4 window may not contain the true
    # argmax: approximate spread below 10x the measured f32r error bound
    # (end-to-end |approx/N - exact| <= 4.9e-4 * top1 scale on this
    # distribution) -> exact argmax over all 224 lags instead.
    scale = np.abs(val8[:, :, 0]) + 1e-20
    spread = val8[:, :, 0] - val8[:, :, N_SLOTS - 1]
    risky = spread < 5e-3 * scale
    if np.any(risky):
        rb, rf = np.nonzero(risky)
        best_idx[rb, rf] = _full_rescore(xpad, rb, rf)

    period = best_idx.astype(np.float32) + np.float32(MIN_PERIOD)
    f0 = np.float32(SR) / (period + np.float32(1e-8))
    return np.clip(f0, np.float32(50.0), np.float32(500.0)).astype(np.float32)
